# revision 36
# baseline (speedup 1.0000x reference)
"""Trainium2 Bass kernel for a BasicTransformerBlock (self-attn + cross-attn +
GeGLU FF), data-parallel over (batch, seq-half) across 8 NeuronCores.

Sharding: core c handles batch b=c//2, sequence half h=c%2 (1024 query rows).
K/V for self-attention need the full 2048-row sequence of the batch, so each
core computes full-sequence K/V locally (duplicated with its pair core) --
cheaper than any cross-core exchange at these sizes. No collectives.

On-chip layout strategy:
- residual x stays token-major fp32 [128p, 8sc, 1024D] (exact residual stream)
- layernorm per 128-token chunk via bn_stats (free-dim reduce), output cast to
  bf16 and DMA-transposed (XBAR) into feature-major xnT [128p(D), dc, S] which
  feeds all matmuls without further transposes
- all matmuls bf16 x bf16 -> fp32 PSUM; weights are cast to bf16 on the fly
- attention computed transposed: S^T[k,q] tiles so softmax sum over k is done
  by an appended ones-column in V (row 64 of the AV PSUM = sumexp);
  max-subtraction is skipped (|scores| <= ~3 at this model's weight scale)
- each stage's output projection uses the activation tile as the stationary
  operand, so stage outputs land token-major for the fp32 residual add
"""
import contextlib

import numpy as np

import concourse.bass as bass
import concourse.tile as tile
from concourse import bacc, mybir
from concourse import bass_utils

F32 = mybir.dt.float32
BF16 = mybir.dt.bfloat16
AF = mybir.ActivationFunctionType
OP = mybir.AluOpType

B, S, D = 4, 2048, 1024
CTX_LEN, CTX_DIM = 77, 768
H, DH = 16, 64
INNER = H * DH            # 1024
FF = D * 4                # 4096
FF2 = FF * 2              # 8192
EPS = 1e-5
P = 128
Q = 1024                  # own query rows per core
SCO = Q // P              # 8  own seq chunks
SCF = S // P              # 16 full seq chunks
DC = D // P               # 8  feature chunks
ICH = INNER // P          # 8  inner chunks
KC = S // P               # 16 key chunks (self)
NQH = Q // 512            # 2  query 512-halves
SCALE = DH ** -0.5        # 0.125

_CACHED = {}


def _ln_chunk(nc, pool, xc_f32, out_bf_chunk, eps_t, g_bc=None, b_bc=None):
    """LayerNorm (x-mu)*rstd*g+b for one [128, D] token chunk -> bf16 out.
    g/b are pre-broadcast [128, D] bf16 tiles (avoids a post-transpose
    all-chunks barrier)."""
    stats = pool.tile([P, 2, 6], F32, tag="lnstats")
    nc.vector.bn_stats(stats[:, 0, :], xc_f32[:, 0:512])
    nc.vector.bn_stats(stats[:, 1, :], xc_f32[:, 512:1024])
    mv = pool.tile([P, 2], F32, tag="lnmv")
    nc.vector.bn_aggr(mv[:], stats[:])
    rstd = pool.tile([P, 1], F32, tag="lnrstd")
    nc.scalar.activation(rstd[:], mv[:, 1:2], AF.Sqrt, bias=eps_t[:])
    nc.vector.reciprocal(rstd[:], rstd[:])
    nmr = pool.tile([P, 1], F32, tag="lnnmr")
    nc.vector.tensor_scalar(nmr[:], mv[:, 0:1], rstd[:, 0:1], -1.0,
                            op0=OP.mult, op1=OP.mult)
    nc.scalar.activation(out_bf_chunk, xc_f32, AF.Identity,
                         bias=nmr[:, 0:1], scale=rstd[:, 0:1])
    if g_bc is not None:
        nc.vector.tensor_tensor(out_bf_chunk, out_bf_chunk, g_bc[:],
                                OP.mult)
        nc.vector.tensor_tensor(out_bf_chunk, out_bf_chunk, b_bc[:], OP.add)


def _gb_broadcast(nc, pool, wp, g_d, b_d):
    """Materialize LN g/b as [128, D] bf16 broadcast tiles."""
    out = []
    for tg, w in (("gbc_g", g_d), ("gbc_b", b_d)):
        stg = wp.tile([P, D], F32, tag="w2f32", name="gbstg", bufs=2)
        src = w.ap()
        bsrc = bass.AP(tensor=src.tensor, offset=src.offset,
                       ap=[[0, P], [1, D]])
        nc.gpsimd.dma_start(stg[:], bsrc)
        t = pool.tile([P, D], BF16, tag=tg, name=tg)
        nc.vector.tensor_copy(t[:], stg[:])
        out.append(t)
    return out


def _load_w_bf(nc, wp, out_bf, dram_ap3, n_dc, col_lo, col_n,
               use_act=False):
    """DMA f32 weight slice [(dc p), cols] -> staging -> cast into out_bf
    [128, n_dc, col_n], in column quarters to keep staging small."""
    step = 256
    off = 0
    while off < col_n:
        w = min(step, col_n - off)
        stg = wp.tile([P, n_dc, step], F32, tag="wf32", bufs=2)
        nc.sync.dma_start(stg[:, :, 0:w],
                          dram_ap3[:, 0:n_dc, col_lo + off: col_lo + off + w])
        if use_act:
            nc.scalar.copy(out_bf[:, :, off:off + w], stg[:, :, 0:w])
        else:
            nc.vector.tensor_copy(out_bf[:, :, off:off + w], stg[:, :, 0:w])
        off += w


def _xnt_tiles(pool, n_cols, tagp):
    return [pool.tile([P, DC, 512], BF16, tag=f"{tagp}{i}", name=f"{tagp}{i}")
            for i in range(n_cols // 512)]


def _xnt(tiles, dc, lo, n):
    t = lo // 512
    assert lo + n <= (t + 1) * 512
    return tiles[t][:, dc, lo - t * 512: lo - t * 512 + n]


def _attn_normalize(nc, recp, psp, ones64, pso, dst_ap, o):
    """dst = pso[0:64,:] * (1/pso[64,:]). The unnormalized rows and the
    sumexp reciprocal are pulled out of PSUM immediately so the AV bank
    frees for the next head; the broadcast/multiply tail runs off-path.
    The rank-1 broadcast lands at partition offset o because walrus
    requires equal base partitions when both TensorTensor inputs are
    SBUF."""
    rec = recp.tile([1, 512], F32, tag="recf")
    nc.vector.reciprocal(rec[:], pso[64:65, :])
    nc.vector.tensor_copy(dst_ap, pso[0:64, :])
    recb = recp.tile([1, 512], BF16, tag="recb")
    nc.vector.tensor_copy(recb[:], rec[:])
    psr = psp.tile([P, 512], F32, tag="psB", name="psr")
    nc.tensor.matmul(psr[o:o + 64, :], ones64[:], recb[:],
                     start=True, stop=True)
    rsb = recp.tile([P, 512], BF16, tag="recsb", name="rsb")
    nc.vector.tensor_copy(rsb[o:o + 64, :], psr[o:o + 64, :])
    nc.vector.tensor_tensor(dst_ap, dst_ap, rsb[o:o + 64, :], OP.mult)


def build_nc():
    nc = bacc.Bacc("TRN2", target_bir_lowering=False, debug=False,
                   num_devices=8, enable_asserts=False)

    xfull_d = nc.dram_tensor("x_full", [S, D], F32, kind="ExternalInput")
    ctxT_d = nc.dram_tensor("ctxT", [CTX_DIM, P], F32, kind="ExternalInput")
    w_d = {}
    for nm, shp in [("ln1_g", [D]), ("ln1_b", [D]), ("ln2_g", [D]),
                    ("ln2_b", [D]), ("ln3_g", [D]), ("ln3_b", [D]),
                    ("a1_wq", [D, INNER]), ("a1_wk", [D, INNER]),
                    ("a1_wv", [D, INNER]), ("a1_wo", [INNER, D]),
                    ("a1_bo", [D]),
                    ("a2_wq", [D, INNER]), ("a2_wk", [CTX_DIM, INNER]),
                    ("a2_wv", [CTX_DIM, INNER]), ("a2_wo", [INNER, D]),
                    ("a2_bo", [D]),
                    ("ff_w1", [D, FF2]), ("ff_b1", [FF2]),
                    ("ff_w2", [FF, D]), ("ff_b2", [D])]:
        w_d[nm] = nc.dram_tensor(nm, shp, F32, kind="ExternalInput")
    out_d = nc.dram_tensor("out", [Q, D], F32, kind="ExternalOutput")

    xfull_r = xfull_d.ap().rearrange("(sc p) d -> p sc d", p=P)
    out_r = out_d.ap().rearrange("(sc p) d -> p sc d", p=P)
    wq1_r = w_d["a1_wq"].ap().rearrange("(dc p) i -> p dc i", p=P)
    wk1_r = w_d["a1_wk"].ap().rearrange("(dc p) i -> p dc i", p=P)
    wv1_r = w_d["a1_wv"].ap().rearrange("(dc p) i -> p dc i", p=P)
    wo1_r = w_d["a1_wo"].ap().rearrange("(ic p) d -> p ic d", p=P)
    wq2_r = w_d["a2_wq"].ap().rearrange("(dc p) i -> p dc i", p=P)
    wk2_r = w_d["a2_wk"].ap().rearrange("(dc p) i -> p dc i", p=P)
    wv2_r = w_d["a2_wv"].ap().rearrange("(dc p) i -> p dc i", p=P)
    wo2_r = w_d["a2_wo"].ap().rearrange("(ic p) d -> p ic d", p=P)
    w1_r = w_d["ff_w1"].ap().rearrange("(dc p) f -> p dc f", p=P)
    w2_r = w_d["ff_w2"].ap().rearrange("(ft p) d -> p ft d", p=P)
    ctxT_r = ctxT_d.ap().rearrange("(dc p) s -> p dc s", p=P)

    with tile.TileContext(nc) as tc, contextlib.ExitStack() as ctx:
        pers = ctx.enter_context(tc.tile_pool(name="pers", bufs=1))
        wp = ctx.enter_context(tc.tile_pool(name="wp", bufs=1))
        wbfp = ctx.enter_context(tc.tile_pool(name="wbfp", bufs=2))
        lnp = ctx.enter_context(tc.tile_pool(name="lnp", bufs=4))
        recp = ctx.enter_context(tc.tile_pool(name="recp", bufs=2))
        psp = ctx.enter_context(tc.tile_pool(name="psp", bufs=2, space="PSUM"))

        # ---- persistent staging ----
        eps_t = pers.tile([P, 1], F32)
        nc.vector.memset(eps_t[:], EPS)
        ones64 = pers.tile([1, 64], BF16)
        nc.vector.memset(ones64[:], 1.0)

        ones128 = pers.tile([1, P], BF16)
        nc.vector.memset(ones128[:], 1.0)
        bias_row = {}
        for nm in ["a1_bo", "a2_bo", "ff_b2"]:
            tf = wp.tile([1, D], F32, tag="w2f32", name="brf", bufs=2)
            nc.sync.dma_start(tf[:], w_d[nm].ap().rearrange("(a d) -> a d", a=1))
            t = pers.tile([1, D], BF16, tag=f"br_{nm}")
            nc.vector.tensor_copy(t[:], tf[:])
            bias_row[nm] = t
        ffb1 = pers.tile([P, FF2 // P], F32)
        nc.sync.dma_start(ffb1[:], w_d["ff_b1"].ap().rearrange(
            "(fc p) -> p fc", p=P))

        # ---- residual x (own rows, token-major fp32); loads are emitted
        # inside stage 1 after the first weight so the DMA queue order
        # matches consumption order ----
        x_sb = pers.tile([P, SCO, D], F32)

        # =========================================================
        # Stage 1: self-attention
        # =========================================================
        with tc.tile_pool(name="sa", bufs=1) as sa, \
             tc.tile_pool(name="sax", bufs=2) as sax, \
             tc.tile_pool(name="saq", bufs=6) as saq:
            # LN g/b broadcast first (feeds every LN chunk), then phase-0
            # wq so its DMA+cast overlap the LN chain
            g_bc, b_bc = _gb_broadcast(nc, sa, wp, w_d["ln1_g"],
                                       w_d["ln1_b"])
            wqb0 = wbfp.tile([P, DC, 512], BF16, tag="wbf", name="wqb0")
            _load_w_bf(nc, wp, wqb0, wq1_r, DC, 0, 512)
            for sc in range(SCO):
                nc.sync.dma_start(x_sb[:, sc, :], xfull_r[:, sc, :])
            xn1T = _xnt_tiles(sa, S, "xnT")

            def _ln1_chunk(sc):
                xn_bf = sax.tile([P, D], BF16, tag="xnbf", name="xn_bf")
                if sc < SCO:
                    xc = x_sb[:, sc, :]
                else:
                    xo = sax.tile([P, D], F32, tag="xoth", name="xo")
                    nc.sync.dma_start(xo[:], xfull_r[:, sc, :])
                    xc = xo[:]
                _ln_chunk(nc, lnp, xc, xn_bf[:], eps_t, g_bc, b_bc)
                nc.sync.dma_start_transpose(
                    _xnt(xn1T, slice(None), sc * P, P), xn_bf[:])

            def _emit_qt(wqb, QT):
                for ic in range(4):
                    for qh in range(NQH):
                        ps = psp.tile([P, 512], F32, tag="psA", name="psq")
                        for dc in range(DC):
                            nc.tensor.matmul(
                                ps[:], wqb[:, dc, ic * P:(ic + 1) * P],
                                _xnt(xn1T, dc, qh * 512, 512),
                                start=(dc == 0), stop=(dc == DC - 1))
                        nc.vector.tensor_scalar_mul(
                            QT[:, ic, qh * 512:(qh + 1) * 512], ps[:], SCALE)

            # own-half LN, then QT(ph0) immediately (needs only own cols),
            # prefetch wk(ph0), then other-half LN
            for sc in range(SCO):
                _ln1_chunk(sc)
            QT0 = sa.tile([P, 4, Q], BF16, tag="qt", name="QT0")
            _emit_qt(wqb0, QT0)
            wkb0 = wbfp.tile([P, DC, 512], BF16, tag="wbf", name="wkb0")
            _load_w_bf(nc, wp, wkb0, wk1_r, DC, 0, 512)
            for sc in range(SCO, SCF):
                _ln1_chunk(sc)

            OT = sa.tile([P, ICH, Q], BF16, tag="ot")

            for ph in range(2):          # head phases: heads 8ph..8ph+7
                i0 = ph * 512            # inner col offset
                if ph == 0:
                    QT = QT0
                    wkb = wkb0
                else:
                    wqb = wbfp.tile([P, DC, 512], BF16, tag="wbf")
                    _load_w_bf(nc, wp, wqb, wq1_r, DC, i0, 512)
                    QT = sa.tile([P, 4, Q], BF16, tag="qt")
                    _emit_qt(wqb, QT)
                    wkb = wbfp.tile([P, DC, 512], BF16, tag="wbf")
                    _load_w_bf(nc, wp, wkb, wk1_r, DC, i0, 512)
                KT = sa.tile([P, 4, S], BF16, tag="kt")
                for ic in range(4):
                    for ks in range(S // 512):
                        ps = psp.tile([P, 512], F32, tag="psA")
                        for dc in range(DC):
                            nc.tensor.matmul(
                                ps[:], wkb[:, dc, ic * P:(ic + 1) * P],
                                _xnt(xn1T, dc, ks * 512, 512),
                                start=(dc == 0), stop=(dc == DC - 1))
                        nc.vector.tensor_copy(
                            KT[:, ic, ks * 512:(ks + 1) * 512], ps[:])
                # V natural [k, inner-slice] with ones column per head
                wvb = wbfp.tile([P, DC, 512], BF16, tag="wbf")
                _load_w_bf(nc, wp, wvb, wv1_r, DC, i0, 512)
                VA = sa.tile([P, KC, 8 * 65], BF16, tag="vt")
                for kc in range(KC):
                    ps = psp.tile([P, 512], F32, tag="psB")
                    for dc in range(DC):
                        nc.tensor.matmul(
                            ps[:], _xnt(xn1T, dc, kc * P, P),
                            wvb[:, dc, :],
                            start=(dc == 0), stop=(dc == DC - 1))
                    dst = VA[:, kc, :].rearrange("p (hl c) -> p hl c",
                                                 c=65)[:, :, 0:64]
                    nc.vector.tensor_copy(dst, ps[:].rearrange(
                        "p (hl c) -> p hl c", c=64))
                ones_cols = VA[:].rearrange(
                    "p kc (hl c) -> p kc hl c", c=65)[:, :, :, 64:65]
                nc.vector.memset(ones_cols, 1.0)

                # attention per head; AV interleaved with scores per k-chunk
                for hl in range(8):
                    ic, o = hl // 2, (hl % 2) * 64
                    pso = [psp.tile([P, 512], F32, tag="psC",
                                    name=f"pso{qh}")
                           for qh in range(NQH)]
                    for kc in range(KC):
                        es = saq.tile([P, Q], BF16, tag="exps")
                        pst = psp.tile([P, Q], F32, tag="psA", name="psts")
                        for qh in range(NQH):
                            nc.tensor.matmul(
                                pst[:, qh * 512:(qh + 1) * 512],
                                KT[o:o + 64, ic, kc * P:(kc + 1) * P],
                                QT[o:o + 64, ic, qh * 512:(qh + 1) * 512],
                                start=True, stop=True)
                        nc.scalar.activation(es[:], pst[:], AF.Exp)
                        for qh in range(NQH):
                            nc.tensor.matmul(
                                pso[qh][0:65, :],
                                VA[:, kc, hl * 65:(hl + 1) * 65],
                                es[:, qh * 512:(qh + 1) * 512],
                                start=(kc == 0), stop=(kc == KC - 1))
                    for qh in range(NQH):
                        _attn_normalize(
                            nc, recp, psp, ones64, pso[qh],
                            OT[o:o + 64, ph * 4 + ic,
                               qh * 512:(qh + 1) * 512], o)

            # output projection + residual + bias
            for ds in range(2):
                wob = wbfp.tile([P, ICH, 512], BF16, tag="wbf")
                _load_w_bf(nc, wp, wob, wo1_r, ICH, ds * 512, 512)
                for sc in range(SCO):
                    ps = psp.tile([P, 512], F32,
                                  tag=("psA" if sc % 2 == 0 else "psB"))
                    for ic in range(ICH):
                        nc.tensor.matmul(
                            ps[:], OT[:, ic, sc * P:(sc + 1) * P],
                            wob[:, ic, :],
                            start=(ic == 0), stop=False)
                    nc.tensor.matmul(
                        ps[:], ones128[:],
                        bias_row["a1_bo"][:, ds * 512:(ds + 1) * 512],
                        start=False, stop=True)
                    xs = x_sb[:, sc, ds * 512:(ds + 1) * 512]
                    nc.vector.tensor_tensor(xs, ps[:], xs, OP.add)

        # =========================================================
        # Stage 2: cross-attention (context 77 tokens, no LN on ctx)
        # =========================================================
        with tc.tile_pool(name="ca", bufs=1) as ca, \
             tc.tile_pool(name="cax", bufs=2) as cax, \
             tc.tile_pool(name="caq", bufs=3) as caq:
            g_bc, b_bc = _gb_broadcast(nc, ca, wp, w_d["ln2_g"],
                                       w_d["ln2_b"])
            wqb2_0 = wbfp.tile([P, DC, 512], BF16, tag="wbf", name="wqb2_0")
            _load_w_bf(nc, wp, wqb2_0, wq2_r, DC, 0, 512, use_act=True)
            xn2T = _xnt_tiles(ca, Q, "xnT")
            for sc in range(SCO):
                xn_bf = cax.tile([P, D], BF16, tag="xnbf")
                _ln_chunk(nc, lnp, x_sb[:, sc, :], xn_bf[:], eps_t, g_bc,
                          b_bc)
                nc.sync.dma_start_transpose(
                    _xnt(xn2T, slice(None), sc * P, P), xn_bf[:])

            # context (host pre-transposed, zero-padded to 128 cols)
            NDC2 = CTX_DIM // P      # 6
            ctxT = ca.tile([P, NDC2, P], BF16, tag="ctxT")
            stg = wp.tile([P, NDC2, P], F32, tag="wf32", bufs=2)
            nc.sync.dma_start(stg[:], ctxT_r[:])
            nc.vector.tensor_copy(ctxT[:], stg[:])

            QcT = ca.tile([P, ICH, Q], BF16, tag="qt")
            KcT = ca.tile([P, ICH, CTX_LEN], BF16, tag="kt")
            VcA = ca.tile([P, H, 65], BF16, tag="vt")
            for ih in range(2):
                i0 = ih * 512
                if ih == 0:
                    wqb = wqb2_0
                else:
                    wqb = wbfp.tile([P, DC, 512], BF16, tag="wbf")
                    _load_w_bf(nc, wp, wqb, wq2_r, DC, i0, 512, use_act=True)
                for ic in range(4):
                    for qh in range(NQH):
                        ps = psp.tile([P, 512], F32, tag="psA")
                        for dc in range(DC):
                            nc.tensor.matmul(
                                ps[:], wqb[:, dc, ic * P:(ic + 1) * P],
                                _xnt(xn2T, dc, qh * 512, 512),
                                start=(dc == 0), stop=(dc == DC - 1))
                        nc.scalar.mul(
                            QcT[:, ih * 4 + ic, qh * 512:(qh + 1) * 512],
                            ps[:], SCALE)
                wkb = wbfp.tile([P, NDC2, 512], BF16, tag="wbf")
                _load_w_bf(nc, wp, wkb, wk2_r, NDC2, i0, 512, use_act=True)
                for ic in range(4):
                    ps = psp.tile([P, 512], F32, tag="psB")
                    for dc in range(NDC2):
                        nc.tensor.matmul(
                            ps[:, 0:CTX_LEN],
                            wkb[:, dc, ic * P:(ic + 1) * P],
                            ctxT[:, dc, 0:CTX_LEN],
                            start=(dc == 0), stop=(dc == NDC2 - 1))
                    nc.scalar.copy(KcT[:, ih * 4 + ic, :],
                                   ps[:, 0:CTX_LEN])
                wvb = wbfp.tile([P, NDC2, 512], BF16, tag="wbf")
                _load_w_bf(nc, wp, wvb, wv2_r, NDC2, i0, 512, use_act=True)
                ps = psp.tile([P, 512], F32, tag="psB")
                for dc in range(NDC2):
                    nc.tensor.matmul(ps[:], ctxT[:, dc, :], wvb[:, dc, :],
                                     start=(dc == 0), stop=(dc == NDC2 - 1))
                dst = VcA[:, ih * 8:(ih + 1) * 8, 0:64]
                nc.scalar.copy(dst, ps[:].rearrange(
                    "p (hl c) -> p hl c", c=64))
            nc.vector.memset(VcA[:, :, 64:65], 1.0)

            # Unnormalized O^T is copied out of PSUM immediately (ACT) so
            # the AV PSUM slot frees for the next head; the normalization
            # tail (reciprocal -> broadcast-matmul -> in-place multiply)
            # runs off the critical chain with deep stage-local buffers.
            OcT = ca.tile([P, ICH, Q], BF16, tag="ot")
            for h in range(H):
                ic, o = h // 2, (h % 2) * 64
                es = caq.tile([P, Q], BF16, tag="exps")
                for qh in range(NQH):
                    pss = psp.tile([P, 512], F32, tag="psA")
                    nc.tensor.matmul(
                        pss[0:CTX_LEN, :], KcT[o:o + 64, ic, :],
                        QcT[o:o + 64, ic, qh * 512:(qh + 1) * 512],
                        start=True, stop=True)
                    nc.scalar.activation(
                        es[0:CTX_LEN, qh * 512:(qh + 1) * 512],
                        pss[0:CTX_LEN, :], AF.Exp)
                    pso = psp.tile([P, 512], F32, tag="psC")
                    nc.tensor.matmul(
                        pso[0:65, :], VcA[0:CTX_LEN, h, :],
                        es[0:CTX_LEN, qh * 512:(qh + 1) * 512],
                        start=True, stop=True)
                    dst = OcT[o:o + 64, ic, qh * 512:(qh + 1) * 512]
                    rec = caq.tile([1, 512], F32, tag="recfc", bufs=4,
                                   name="recc")
                    nc.vector.reciprocal(rec[:], pso[64:65, :])
                    nc.scalar.copy(dst, pso[0:64, :])
                    recb = caq.tile([1, 512], BF16, tag="recbc", bufs=4,
                                    name="recbc")
                    nc.vector.tensor_copy(recb[:], rec[:])
                    # broadcast lands at partition offset o so the in-place
                    # SBUF*SBUF multiply has equal base partitions (walrus
                    # requires it when both inputs are SBUF)
                    psr = psp.tile([P, 512], F32, tag="psB", name="psrc")
                    nc.tensor.matmul(psr[o:o + 64, :], ones64[:], recb[:],
                                     start=True, stop=True)
                    rsb = caq.tile([P, 512], BF16, tag="recsbc", bufs=4,
                                   name="rsbc")
                    nc.scalar.copy(rsb[o:o + 64, :], psr[o:o + 64, :])
                    nc.vector.tensor_tensor(dst, dst, rsb[o:o + 64, :],
                                            OP.mult)

            for ds in range(2):
                wob = wbfp.tile([P, ICH, 512], BF16, tag="wbf")
                _load_w_bf(nc, wp, wob, wo2_r, ICH, ds * 512, 512, use_act=True)
                for sc in range(SCO):
                    ps = psp.tile([P, 512], F32,
                                  tag=("psA" if sc % 2 == 0 else "psB"))
                    for ic in range(ICH):
                        nc.tensor.matmul(
                            ps[:], OcT[:, ic, sc * P:(sc + 1) * P],
                            wob[:, ic, :],
                            start=(ic == 0), stop=False)
                    nc.tensor.matmul(
                        ps[:], ones128[:],
                        bias_row["a2_bo"][:, ds * 512:(ds + 1) * 512],
                        start=False, stop=True)
                    xs = x_sb[:, sc, ds * 512:(ds + 1) * 512]
                    nc.vector.tensor_tensor(xs, ps[:], xs, OP.add)

        # =========================================================
        # Stage 3: GeGLU feed-forward
        # =========================================================
        with tc.tile_pool(name="ff", bufs=1) as ff, \
             tc.tile_pool(name="ffx", bufs=2) as ffx:
            def _load_w1(g):
                w1b = ffx.tile([P, DC, 256], BF16, tag="w1bf", bufs=3,
                               name=f"w1b{g}")
                stg = ffx.tile([P, DC, 256], F32, tag="w1f32", bufs=1,
                               name=f"w1stg{g}")
                nc.sync.dma_start(stg[:, :, 0:128],
                                  w1_r[:, :, g * P:(g + 1) * P])
                nc.sync.dma_start(stg[:, :, 128:256],
                                  w1_r[:, :, FF + g * P: FF + (g + 1) * P])
                nc.vector.tensor_copy(w1b[:], stg[:])
                return w1b

            w1_pre = {g: _load_w1(g) for g in range(3)}
            g_bc, b_bc = _gb_broadcast(nc, ff, wp, w_d["ln3_g"],
                                       w_d["ln3_b"])
            xn3T = _xnt_tiles(ff, Q, "xnT")
            for sc in range(SCO):
                xn_bf = ffx.tile([P, D], BF16, tag="xnbf")
                _ln_chunk(nc, lnp, x_sb[:, sc, :], xn_bf[:], eps_t, g_bc,
                          b_bc)
                nc.sync.dma_start_transpose(
                    _xnt(xn3T, slice(None), sc * P, P), xn_bf[:])

            gT = ff.tile([P, FF // P, Q], BF16, tag="gt")
            for g in range(FF // P):         # 32 paired (u, gate) chunks
                w1b = w1_pre.pop(g) if g in w1_pre else _load_w1(g)
                gel = ffx.tile([P, Q], BF16, tag="gelu")
                for qh in range(NQH):
                    psu = psp.tile([P, 512], F32, tag="psA")
                    psg = psp.tile([P, 512], F32, tag="psB")
                    for dc in range(DC):
                        nc.tensor.matmul(
                            psu[:], w1b[:, dc, 0:128],
                            _xnt(xn3T, dc, qh * 512, 512),
                            start=(dc == 0), stop=(dc == DC - 1))
                    for dc in range(DC):
                        nc.tensor.matmul(
                            psg[:], w1b[:, dc, 128:256],
                            _xnt(xn3T, dc, qh * 512, 512),
                            start=(dc == 0), stop=(dc == DC - 1))
                    nc.scalar.activation(
                        gel[:, qh * 512:(qh + 1) * 512], psg[:], AF.Gelu,
                        bias=ffb1[:, 32 + g:33 + g])
                    nc.vector.scalar_tensor_tensor(
                        gT[:, g, qh * 512:(qh + 1) * 512], psu[:],
                        ffb1[:, g:g + 1], gel[:, qh * 512:(qh + 1) * 512],
                        op0=OP.add, op1=OP.mult)

            # w2: out token-major, accumulate over 32 ff chunks
            FT = FF // P
            for half in range(2):
                pss = {}
                for si, sc in enumerate(range(half * 4, half * 4 + 4)):
                    if si < 2:
                        w = psp.tile([P, Q], F32, tag="psA",
                                     name=f"pssw_{sc}")
                        for ds in range(2):
                            pss[(sc, ds)] = w[:, ds * 512:(ds + 1) * 512]
                    else:
                        tg = "psB" if si == 2 else "psC"
                        for ds in range(2):
                            pss[(sc, ds)] = psp.tile(
                                [P, 512], F32, tag=tg,
                                name=f"pss_{sc}_{ds}")[:]
                for ft in range(FT):
                    w2b = ffx.tile([P, D], BF16, tag="w2bf", bufs=4)
                    stg = wp.tile([P, D], F32, tag="w2f32", bufs=2)
                    nc.sync.dma_start(stg[:], w2_r[:, ft, :])
                    nc.vector.tensor_copy(w2b[:], stg[:])
                    for sc in range(half * 4, half * 4 + 4):
                        for ds in range(2):
                            nc.tensor.matmul(
                                pss[(sc, ds)][:],
                                gT[:, ft, sc * P:(sc + 1) * P],
                                w2b[:, ds * 512:(ds + 1) * 512],
                                start=(ft == 0), stop=False)
                for sc in range(half * 4, half * 4 + 4):
                    for ds in range(2):
                        nc.tensor.matmul(
                            pss[(sc, ds)][:], ones128[:],
                            bias_row["ff_b2"][:, ds * 512:(ds + 1) * 512],
                            start=False, stop=True)
                        xs = x_sb[:, sc, ds * 512:(ds + 1) * 512]
                        nc.vector.tensor_tensor(xs, pss[(sc, ds)][:], xs,
                                                OP.add)
                    nc.sync.dma_start(out_r[:, sc, :], x_sb[:, sc, :])

    nc.compile()
    return nc


def kernel(**inputs):
    inputs = {k: np.asarray(v, dtype=np.float32) for k, v in inputs.items()}
    if "nc" not in _CACHED:
        _CACHED["nc"] = build_nc()
    nc = _CACHED["nc"]

    x = inputs["x"]
    context = inputs["context"]
    wnames = ["ln1_g", "ln1_b", "ln2_g", "ln2_b", "ln3_g", "ln3_b",
              "a1_wq", "a1_wk", "a1_wv", "a1_wo", "a1_bo",
              "a2_wq", "a2_wk", "a2_wv", "a2_wo", "a2_bo",
              "ff_w1", "ff_b1", "ff_w2", "ff_b2"]
    wmap = {nm: np.ascontiguousarray(inputs[nm]) for nm in wnames}

    in_maps = []
    for c in range(8):
        b, h = c // 2, c % 2
        xb = x[b]
        if h == 1:  # rotate so own rows are 0..1023 (attention is order-inv)
            xb = np.concatenate([xb[Q:], xb[:Q]], axis=0)
        ctxT = np.zeros((CTX_DIM, P), np.float32)
        ctxT[:, :CTX_LEN] = context[b].T
        m = {"x_full": np.ascontiguousarray(xb), "ctxT": ctxT}
        m.update(wmap)
        in_maps.append(m)

    res = bass_utils.run_bass_kernel_spmd(nc, in_maps, core_ids=list(range(8)))
    out = np.empty((B, S, D), np.float32)
    for c in range(8):
        b, h = c // 2, c % 2
        out[b, h * Q:(h + 1) * Q, :] = res.results[c]["out"]
    return out
